# revision 20
# baseline (speedup 1.0000x reference)
"""Trainium2 Bass kernel for nn_Conv2DSum (logconv1x1_2d / SPN sum layer).

Math: out[b,h,w,s] = logsumexp_c( x[b,h,w,c] + log_softmax(acc)[c,s] )
Since w = softmax(acc) along c sums to 1, the result equals
    out = log( exp(x) @ w )
which is a convex combination of exp(x_c) — numerically safe in fp16/fp32
range for N(0,1)-scale inputs (no max-subtraction needed).

V7 strategy (per core, batch-sharded 8 ways: 65536 rows x 32 ch),
memory-regime:
  - All HBM I/O in fp16 (half the bytes of the fp32 baseline; rel-err
    budget 2e-2 is ~100x what fp16 costs here).
  - The host stores x PRE-TRANSPOSED in quad-row blocks:
    xT[u, q] with u = (row%4)*32 + ch, q = row//4, i.e. [128, 16384] per
    core. The device then needs NO transpose at all: the contraction dim
    is already on partitions.
  - exp via DVE bit-trick: i16 = round(x*1024/ln2 + (15-sigma)*1024)
    written as int16; those bits ARE ~exp(x) in fp16 (max ~4% rel,
    deterministic, mostly averaged out by the 32-ch weighted sum). One
    tensor_scalar per tile, SBUF fp16 -> SBUF int16. Keeps ACT free for
    the exact Ln.
  - One fp16 matmul per [128,128] slice: stationary = p~ (bitcast fp16),
    moving = block-diag weight (4 copies of the 32x32 softmax matrix), so
    4 quad-rows resolve per matmul: psO[q, g*32+s] = sum_c p~[g*32+c,q]w[c,s].
  - Exact Ln via ScalarE ACT, whole PSUM group [128,2048] -> SBUF fp16.
  - out fp16 [128, 16384] (quad-row-major like the input); host widens and
    re-permutes to [B,H,W,S].

End-to-end rel err ~8.2e-3 vs the 2e-2 gate, dominated by the exp
bit-trick; exact-exp fallback (USE_TRICK_EXP=False) runs exp on ACT
(~1.5e-3 but slower).
"""

from contextlib import ExitStack

import numpy as np

import concourse.bass as bass
import concourse.tile as tile
from concourse import mybir

# Problem shape (hardcoded per contest rules)
B, H, W, C_IN, N_SUMS = 32, 128, 128, 32, 32
N_CORES = 8
B_PER_CORE = B // N_CORES              # 4
ROWS_PER_CORE = B_PER_CORE * H * W     # 65536
TOTAL_FREE = ROWS_PER_CORE // 4        # 16384 quad-rows per core

F32 = mybir.dt.float32
F16 = mybir.dt.float16
I16 = mybir.dt.int16

USE_TRICK_EXP = True

# exp(x) ~= bitcast_fp16(int16(round(A16*x + B16)))
_SIGMA = 0.0455
A16 = 1024.0 / float(np.log(2.0))
B16 = (15.0 - _SIGMA) * 1024.0

# (col_offset, width) of each input DMA; first two small so compute starts
# after ~0.25MB instead of 1MB. All chunks stay resident in SBUF (32KB per
# partition total), so every input DMA issues with no WAR wait and the
# in-order Sync queue never head-of-line blocks an input load behind an
# output store.
X_CHUNKS = [
    (0, 1024), (1024, 1024), (2048, 2048),
    (4096, 4096), (8192, 4096), (12288, 4096),
]
# compute tiles (col, width): small first tiles shorten the lead-in, small
# last tiles shorten the serial DVE->MM->Ln->DMA drain tail.
C_TILES = [(0, 1024), (1024, 1024)] + [
    (c, 2048) for c in range(2048, 14336, 2048)
] + [(14336, 1024), (15360, 1024)]


def build_kernel(nc: bass.Bass, repeat: int = 1):
    x_d = nc.dram_tensor("x", [128, TOTAL_FREE], F16, kind="ExternalInput").ap()
    wblk_d = nc.dram_tensor("w_blk", [128, 128], F16, kind="ExternalInput").ap()
    out_d = nc.dram_tensor("out", [128, TOTAL_FREE], F16, kind="ExternalOutput").ap()

    with tile.TileContext(nc) as tc, ExitStack() as ctx:
        const_pool = ctx.enter_context(tc.tile_pool(name="const", bufs=1))
        # SBUF is ~64KB/partition usable; pool slots are sized to the largest
        # tile, so segregate x chunks by width to stay within budget.
        x_pools = {}
        for width in sorted({w for _, w in X_CHUNKS}):
            n = sum(1 for _, w in X_CHUNKS if w == width)
            x_pools[width] = ctx.enter_context(
                tc.tile_pool(name=f"x{width}", bufs=n)
            )
        p_pool = ctx.enter_context(tc.tile_pool(name="p", bufs=3))
        o_pool = ctx.enter_context(tc.tile_pool(name="o", bufs=3))
        # psO [128,2048] fp32 = 4 banks; 2 bufs = all 8 banks
        psO_pool = ctx.enter_context(tc.tile_pool(name="psO", bufs=2, space="PSUM"))

        def load_chunk(c):
            off, width = X_CHUNKS[c]
            xt = x_pools[width].tile([128, width], F16, tag=f"x{c}")
            nc.sync.dma_start(xt[:], x_d[:, off : off + width])
            return xt

        # x chunk 0 first (it gates the first DVE), then wblk, then the rest.
        xc0 = load_chunk(0)
        wblk = const_pool.tile([128, 128], F16, tag="wblk")
        nc.sync.dma_start(wblk[:], wblk_d)

        dummy_w = const_pool.tile([128, 8], mybir.dt.bfloat16, tag="dummyw")
        nc.gpsimd.memset(dummy_w[:], 1.0)

        # tiny dummy activation up front: forces the ACT table load to
        # overlap the first x DMA instead of sitting on the critical path
        warm_pool = ctx.enter_context(tc.tile_pool(name="warm", bufs=1))
        warm = warm_pool.tile([128, 1], F32, tag="warm")
        nc.scalar.activation(
            warm[:], dummy_w[:, 0:1], mybir.ActivationFunctionType.Ln
        )
        if not USE_TRICK_EXP:
            nc.scalar.activation(
                warm[:], dummy_w[:, 0:1], mybir.ActivationFunctionType.Exp
            )

        def chunk_of(col):
            for ci, (off, width) in enumerate(X_CHUNKS):
                if off <= col < off + width:
                    return ci, col - off
            raise AssertionError(col)

        for _rep in range(repeat):
            chunk_bufs = {0: xc0 if _rep == 0 else load_chunk(0)}
            for ci in range(1, len(X_CHUNKS)):
                chunk_bufs[ci] = load_chunk(ci)

            for col, width in C_TILES:
                ci, coff = chunk_of(col)
                xt = chunk_bufs[ci]
                if USE_TRICK_EXP:
                    pt = p_pool.tile([128, width], I16)
                    nc.vector.tensor_scalar(
                        pt[:],
                        xt[:, coff : coff + width],
                        A16,
                        B16,
                        op0=mybir.AluOpType.mult,
                        op1=mybir.AluOpType.add,
                    )
                    ptv = pt[:].bitcast(F16)
                else:
                    pt = p_pool.tile([128, width], F16)
                    nc.scalar.activation(
                        pt[:],
                        xt[:, coff : coff + width],
                        mybir.ActivationFunctionType.Exp,
                    )
                    ptv = pt[:]
                psO = psO_pool.tile([128, width], F32)
                for j in range(width // 128):
                    nc.tensor.matmul(
                        psO[:, bass.ts(j, 128)],
                        ptv[:, bass.ts(j, 128)],
                        wblk[:],
                        start=(j % 4 == 0),
                        stop=(j % 4 == 3),
                    )
                ot = o_pool.tile([128, width], F16)
                nc.scalar.activation(
                    ot[:], psO[:], mybir.ActivationFunctionType.Ln
                )
                nc.sync.dma_start(out_d[:, col : col + width], ot[:])
    return nc


# walrus rejects >1 embedded sync-wait on engine-instruction structs
# (Matmult/Activation/DMA...). The NX sequencer executes embedded waits in
# stream order anyway, so spilling all-but-one wait onto dedicated nops
# immediately before the instruction is semantically identical.
_SPLIT_TYPES = (
    "InstMatmult",
    "InstLdweights",
    "InstActivation",
    "InstDMACopy",
    "InstMemset",
    "InstTensorTensor",
    "InstTensorScalarPtr",
    "InstCopy",
    "InstTensorReduce",
    "InstDrain",
    "InstNoOp",
)


def _split_embedded_waits(nc: bass.Bass):
    for fn in nc.m.functions:
        for blk in fn.blocks:
            insts = blk.instructions
            out = []
            for inst in insts:
                si = inst.sync_info
                if (
                    si is not None
                    and si.on_wait
                    and len(si.on_wait) > 1
                    and type(inst).__name__ in _SPLIT_TYPES
                ):
                    waits = list(si.on_wait)
                    for i, w in enumerate(waits[:-1]):
                        nop = mybir.InstNoOp(
                            name=f"{inst.name}-sw{i}",
                            engine=inst.engine,
                            sync_info=mybir.SyncInfo(on_wait=[w], on_update=[]),
                            bass_nofuse=True,
                        )
                        out.append(nop)
                    inst.sync_info = mybir.SyncInfo(
                        on_wait=[waits[-1]], on_update=list(si.on_update)
                    )
                out.append(inst)
            if len(out) != len(insts):
                blk.instructions[:] = out


def _host_weights(accumulators: np.ndarray) -> np.ndarray:
    """log_softmax over c of [1,1,Cin,S] accumulators -> exp -> block-diag."""
    acc = np.asarray(accumulators, dtype=np.float64)[0, 0]      # [Cin, S]
    m = acc.max(axis=0, keepdims=True)
    e = np.exp(acc - m)
    w = (e / e.sum(axis=0, keepdims=True)).astype(np.float16)   # [Cin, S]
    w_blk = np.zeros((128, 128), dtype=np.float16)
    for g in range(4):
        w_blk[32 * g : 32 * g + 32, 32 * g : 32 * g + 32] = w
    return w_blk


def make_in_maps(x: np.ndarray, acc: np.ndarray) -> list[dict]:
    x16 = np.asarray(x).astype(np.float16)
    w_blk = _host_weights(np.asarray(acc, dtype=np.float32))
    in_maps = []
    for c in range(N_CORES):
        xc = x16[c * B_PER_CORE : (c + 1) * B_PER_CORE].reshape(-1, C_IN)
        # quad-row transpose: xT[(row%4)*32 + ch, row//4]
        xT = np.ascontiguousarray(
            xc.reshape(TOTAL_FREE, 4, C_IN).transpose(1, 2, 0).reshape(
                128, TOTAL_FREE
            )
        )
        in_maps.append({"x": xT, "w_blk": w_blk})
    return in_maps


def unpack_out(o: np.ndarray) -> np.ndarray:
    """[128, 16384] quad-row-major fp16 -> [B_PER_CORE, H, W, S] fp32."""
    # out[p, t*2048 + j*128 + g*32 + s] holds row 4*(t*2048 + j*128 + p)+g
    o = o.reshape(128, TOTAL_FREE // 128, 4, 32)
    # axes: [p, qblk, g, s] where q = qblk*128 + p, row = 4q + g
    o = o.transpose(1, 0, 2, 3)  # [qblk, p, g, s] -> rows ascending
    return o.reshape(B_PER_CORE, H, W, N_SUMS).astype(np.float32)


_CACHE: dict = {}


def make_bass():
    return bass.Bass("TRN2", debug=False, num_swdge_queues=4)


def get_nc():
    if "nc" not in _CACHE:
        nc = build_kernel(make_bass())
        # HW path only: CoreSim can't digest post-hoc inserted nops
        _split_embedded_waits(nc)
        _CACHE["nc"] = nc
    return _CACHE["nc"]


def kernel(**inputs: np.ndarray) -> np.ndarray:
    from concourse.bass_utils import run_bass_kernel_spmd

    in_maps = make_in_maps(inputs["x"], inputs["accumulators"])
    nc = get_nc()
    res = run_bass_kernel_spmd(nc, in_maps, core_ids=list(range(N_CORES)))
    outs = [unpack_out(np.asarray(res.results[c]["out"])) for c in range(N_CORES)]
    return np.concatenate(outs, axis=0)


# revision 22
# speedup vs baseline: 1.0837x; 1.0837x over previous
"""Trainium2 Bass kernel for nn_Conv2DSum (logconv1x1_2d / SPN sum layer).

Math: out[b,h,w,s] = logsumexp_c( x[b,h,w,c] + log_softmax(acc)[c,s] )
Since w = softmax(acc) along c sums to 1, the result equals
    out = log( exp(x) @ w )
which is a convex combination of exp(x_c) — numerically safe in fp16/fp32
range for N(0,1)-scale inputs (no max-subtraction needed).

V7 strategy (per core, batch-sharded 8 ways: 65536 rows x 32 ch),
memory-regime:
  - All HBM I/O in fp16 (half the bytes of the fp32 baseline; rel-err
    budget 2e-2 is ~100x what fp16 costs here).
  - The host stores x PRE-TRANSPOSED in quad-row blocks:
    xT[u, q] with u = (row%4)*32 + ch, q = row//4, i.e. [128, 16384] per
    core. The device then needs NO transpose at all: the contraction dim
    is already on partitions.
  - exp via DVE bit-trick: i16 = round(x*1024/ln2 + (15-sigma)*1024)
    written as int16; those bits ARE ~exp(x) in fp16 (max ~4% rel,
    deterministic, mostly averaged out by the 32-ch weighted sum). One
    tensor_scalar per tile, SBUF fp16 -> SBUF int16. Keeps ACT free for
    the exact Ln.
  - One fp16 matmul per [128,128] slice: stationary = p~ (bitcast fp16),
    moving = block-diag weight (4 copies of the 32x32 softmax matrix), so
    4 quad-rows resolve per matmul: psO[q, g*32+s] = sum_c p~[g*32+c,q]w[c,s].
  - Exact Ln via ScalarE ACT, whole PSUM group [128,2048] -> SBUF fp16.
  - out fp16 [128, 16384] (quad-row-major like the input); host widens and
    re-permutes to [B,H,W,S].

End-to-end rel err ~8.2e-3 vs the 2e-2 gate, dominated by the exp
bit-trick; exact-exp fallback (USE_TRICK_EXP=False) runs exp on ACT
(~1.5e-3 but slower).
"""

from contextlib import ExitStack

import numpy as np

import concourse.bass as bass
import concourse.tile as tile
from concourse import mybir

# Problem shape (hardcoded per contest rules)
B, H, W, C_IN, N_SUMS = 32, 128, 128, 32, 32
N_CORES = 8
B_PER_CORE = B // N_CORES              # 4
ROWS_PER_CORE = B_PER_CORE * H * W     # 65536
TOTAL_FREE = ROWS_PER_CORE // 4        # 16384 quad-rows per core

F32 = mybir.dt.float32
F16 = mybir.dt.float16
I16 = mybir.dt.int16

USE_TRICK_EXP = True

# exp(x) ~= bitcast_fp16(int16(round(A16*x + B16)))
_SIGMA = 0.0455
A16 = 1024.0 / float(np.log(2.0))
B16 = (15.0 - _SIGMA) * 1024.0

# (col_offset, width) of each input DMA; first two small so compute starts
# after ~0.25MB instead of 1MB. All chunks stay resident in SBUF (32KB per
# partition total), so every input DMA issues with no WAR wait and the
# in-order Sync queue never head-of-line blocks an input load behind an
# output store.
X_CHUNKS = [
    (0, 1024), (1024, 1024), (2048, 2048),
    (4096, 4096), (8192, 4096), (12288, 4096),
]
# compute tiles (col, width): small first tiles shorten the lead-in, small
# last tiles shorten the serial DVE->MM->Ln->DMA drain tail.
C_TILES = [(0, 1024), (1024, 1024)] + [
    (c, 2048) for c in range(2048, 14336, 2048)
] + [(14336, 1024), (15360, 1024)]


def build_kernel(nc: bass.Bass, repeat: int = 1):
    x_d = nc.dram_tensor("x", [128, TOTAL_FREE], F16, kind="ExternalInput").ap()
    wblk_d = nc.dram_tensor("w_blk", [128, 128], F16, kind="ExternalInput").ap()
    out_d = nc.dram_tensor("out", [128, TOTAL_FREE], F16, kind="ExternalOutput").ap()

    with tile.TileContext(nc) as tc, ExitStack() as ctx:
        const_pool = ctx.enter_context(tc.tile_pool(name="const", bufs=1))
        # SBUF is ~64KB/partition usable; pool slots are sized to the largest
        # tile, so segregate x chunks by width to stay within budget.
        x_pools = {}
        for width in sorted({w for _, w in X_CHUNKS}):
            n = sum(1 for _, w in X_CHUNKS if w == width)
            x_pools[width] = ctx.enter_context(
                tc.tile_pool(name=f"x{width}", bufs=n)
            )
        p_pool = ctx.enter_context(tc.tile_pool(name="p", bufs=2))
        o_pool = ctx.enter_context(tc.tile_pool(name="o", bufs=4))
        # psO [128,2048] fp32 = 4 banks; 2 bufs = all 8 banks
        psO_pool = ctx.enter_context(tc.tile_pool(name="psO", bufs=2, space="PSUM"))

        def load_chunk(c):
            off, width = X_CHUNKS[c]
            xt = x_pools[width].tile([128, width], F16, tag=f"x{c}")
            nc.sync.dma_start(xt[:], x_d[:, off : off + width])
            return xt

        # x chunk 0 first (it gates the first DVE), then wblk, then the rest.
        xc0 = load_chunk(0)
        wblk = const_pool.tile([128, 128], F16, tag="wblk")
        nc.sync.dma_start(wblk[:], wblk_d)

        dummy_w = const_pool.tile([128, 8], mybir.dt.bfloat16, tag="dummyw")
        nc.gpsimd.memset(dummy_w[:], 1.0)

        # tiny dummy activation up front: forces the ACT table load to
        # overlap the first x DMA instead of sitting on the critical path
        warm_pool = ctx.enter_context(tc.tile_pool(name="warm", bufs=1))
        warm = warm_pool.tile([128, 1], F32, tag="warm")
        nc.scalar.activation(
            warm[:], dummy_w[:, 0:1], mybir.ActivationFunctionType.Ln
        )
        if not USE_TRICK_EXP:
            nc.scalar.activation(
                warm[:], dummy_w[:, 0:1], mybir.ActivationFunctionType.Exp
            )

        def chunk_of(col):
            for ci, (off, width) in enumerate(X_CHUNKS):
                if off <= col < off + width:
                    return ci, col - off
            raise AssertionError(col)

        for _rep in range(repeat):
            chunk_bufs = {0: xc0 if _rep == 0 else load_chunk(0)}
            for ci in range(1, len(X_CHUNKS)):
                chunk_bufs[ci] = load_chunk(ci)

            for col, width in C_TILES:
                ci, coff = chunk_of(col)
                xt = chunk_bufs[ci]
                if USE_TRICK_EXP:
                    pt = p_pool.tile([128, width], I16)
                    nc.vector.tensor_scalar(
                        pt[:],
                        xt[:, coff : coff + width],
                        A16,
                        B16,
                        op0=mybir.AluOpType.mult,
                        op1=mybir.AluOpType.add,
                    )
                    ptv = pt[:].bitcast(F16)
                else:
                    pt = p_pool.tile([128, width], F16)
                    nc.scalar.activation(
                        pt[:],
                        xt[:, coff : coff + width],
                        mybir.ActivationFunctionType.Exp,
                    )
                    ptv = pt[:]
                psO = psO_pool.tile([128, width], F32)
                for j in range(width // 128):
                    nc.tensor.matmul(
                        psO[:, bass.ts(j, 128)],
                        ptv[:, bass.ts(j, 128)],
                        wblk[:],
                        start=(j % 4 == 0),
                        stop=(j % 4 == 3),
                    )
                ot = o_pool.tile([128, width], F16)
                nc.scalar.activation(
                    ot[:], psO[:], mybir.ActivationFunctionType.Ln
                )
                # out stores go on the second HWDGE queue (ACT engine): on a
                # single queue the first store's packets sit behind the whole
                # input backlog (FIFO), which stalled ln_t3+ on ot reuse.
                nc.scalar.dma_start(out_d[:, col : col + width], ot[:])
    return nc


# walrus rejects >1 embedded sync-wait on engine-instruction structs
# (Matmult/Activation/DMA...). The NX sequencer executes embedded waits in
# stream order anyway, so spilling all-but-one wait onto dedicated nops
# immediately before the instruction is semantically identical.
_SPLIT_TYPES = (
    "InstMatmult",
    "InstLdweights",
    "InstActivation",
    "InstDMACopy",
    "InstMemset",
    "InstTensorTensor",
    "InstTensorScalarPtr",
    "InstCopy",
    "InstTensorReduce",
    "InstDrain",
    "InstNoOp",
)


def _split_embedded_waits(nc: bass.Bass):
    for fn in nc.m.functions:
        for blk in fn.blocks:
            insts = blk.instructions
            out = []
            for inst in insts:
                si = inst.sync_info
                if (
                    si is not None
                    and si.on_wait
                    and len(si.on_wait) > 1
                    and type(inst).__name__ in _SPLIT_TYPES
                ):
                    waits = list(si.on_wait)
                    for i, w in enumerate(waits[:-1]):
                        nop = mybir.InstNoOp(
                            name=f"{inst.name}-sw{i}",
                            engine=inst.engine,
                            sync_info=mybir.SyncInfo(on_wait=[w], on_update=[]),
                            bass_nofuse=True,
                        )
                        out.append(nop)
                    inst.sync_info = mybir.SyncInfo(
                        on_wait=[waits[-1]], on_update=list(si.on_update)
                    )
                out.append(inst)
            if len(out) != len(insts):
                blk.instructions[:] = out


def _host_weights(accumulators: np.ndarray) -> np.ndarray:
    """log_softmax over c of [1,1,Cin,S] accumulators -> exp -> block-diag."""
    acc = np.asarray(accumulators, dtype=np.float64)[0, 0]      # [Cin, S]
    m = acc.max(axis=0, keepdims=True)
    e = np.exp(acc - m)
    w = (e / e.sum(axis=0, keepdims=True)).astype(np.float16)   # [Cin, S]
    w_blk = np.zeros((128, 128), dtype=np.float16)
    for g in range(4):
        w_blk[32 * g : 32 * g + 32, 32 * g : 32 * g + 32] = w
    return w_blk


def make_in_maps(x: np.ndarray, acc: np.ndarray) -> list[dict]:
    x16 = np.asarray(x).astype(np.float16)
    w_blk = _host_weights(np.asarray(acc, dtype=np.float32))
    in_maps = []
    for c in range(N_CORES):
        xc = x16[c * B_PER_CORE : (c + 1) * B_PER_CORE].reshape(-1, C_IN)
        # quad-row transpose: xT[(row%4)*32 + ch, row//4]
        xT = np.ascontiguousarray(
            xc.reshape(TOTAL_FREE, 4, C_IN).transpose(1, 2, 0).reshape(
                128, TOTAL_FREE
            )
        )
        in_maps.append({"x": xT, "w_blk": w_blk})
    return in_maps


def unpack_out(o: np.ndarray) -> np.ndarray:
    """[128, 16384] quad-row-major fp16 -> [B_PER_CORE, H, W, S] fp32."""
    # out[p, t*2048 + j*128 + g*32 + s] holds row 4*(t*2048 + j*128 + p)+g
    o = o.reshape(128, TOTAL_FREE // 128, 4, 32)
    # axes: [p, qblk, g, s] where q = qblk*128 + p, row = 4q + g
    o = o.transpose(1, 0, 2, 3)  # [qblk, p, g, s] -> rows ascending
    return o.reshape(B_PER_CORE, H, W, N_SUMS).astype(np.float32)


_CACHE: dict = {}


def make_bass():
    return bass.Bass("TRN2", debug=False, num_swdge_queues=4)


def get_nc():
    if "nc" not in _CACHE:
        nc = build_kernel(make_bass())
        # HW path only: CoreSim can't digest post-hoc inserted nops
        _split_embedded_waits(nc)
        _CACHE["nc"] = nc
    return _CACHE["nc"]


def kernel(**inputs: np.ndarray) -> np.ndarray:
    from concourse.bass_utils import run_bass_kernel_spmd

    in_maps = make_in_maps(inputs["x"], inputs["accumulators"])
    nc = get_nc()
    res = run_bass_kernel_spmd(nc, in_maps, core_ids=list(range(N_CORES)))
    outs = [unpack_out(np.asarray(res.results[c]["out"])) for c in range(N_CORES)]
    return np.concatenate(outs, axis=0)


# revision 24
# speedup vs baseline: 1.1031x; 1.0179x over previous
"""Trainium2 Bass kernel for nn_Conv2DSum (logconv1x1_2d / SPN sum layer).

Math: out[b,h,w,s] = logsumexp_c( x[b,h,w,c] + log_softmax(acc)[c,s] )
Since w = softmax(acc) along c sums to 1, the result equals
    out = log( exp(x) @ w )
which is a convex combination of exp(x_c) — numerically safe in fp16/fp32
range for N(0,1)-scale inputs (no max-subtraction needed).

V7 strategy (per core, batch-sharded 8 ways: 65536 rows x 32 ch),
memory-regime:
  - All HBM I/O in fp16 (half the bytes of the fp32 baseline; rel-err
    budget 2e-2 is ~100x what fp16 costs here).
  - The host stores x PRE-TRANSPOSED in quad-row blocks:
    xT[u, q] with u = (row%4)*32 + ch, q = row//4, i.e. [128, 16384] per
    core. The device then needs NO transpose at all: the contraction dim
    is already on partitions.
  - exp via DVE bit-trick: i16 = round(x*1024/ln2 + (15-sigma)*1024)
    written as int16; those bits ARE ~exp(x) in fp16 (max ~4% rel,
    deterministic, mostly averaged out by the 32-ch weighted sum). One
    tensor_scalar per tile, SBUF fp16 -> SBUF int16. Keeps ACT free for
    the exact Ln.
  - One fp16 matmul per [128,128] slice: stationary = p~ (bitcast fp16),
    moving = block-diag weight (4 copies of the 32x32 softmax matrix), so
    4 quad-rows resolve per matmul: psO[q, g*32+s] = sum_c p~[g*32+c,q]w[c,s].
  - Exact Ln via ScalarE ACT, whole PSUM group [128,2048] -> SBUF fp16.
  - out fp16 [128, 16384] (quad-row-major like the input); host widens and
    re-permutes to [B,H,W,S].

End-to-end rel err ~8.2e-3 vs the 2e-2 gate, dominated by the exp
bit-trick; exact-exp fallback (USE_TRICK_EXP=False) runs exp on ACT
(~1.5e-3 but slower).
"""

from contextlib import ExitStack

import numpy as np

import concourse.bass as bass
import concourse.tile as tile
from concourse import mybir

# Problem shape (hardcoded per contest rules)
B, H, W, C_IN, N_SUMS = 32, 128, 128, 32, 32
N_CORES = 8
B_PER_CORE = B // N_CORES              # 4
ROWS_PER_CORE = B_PER_CORE * H * W     # 65536
TOTAL_FREE = ROWS_PER_CORE // 4        # 16384 quad-rows per core

F32 = mybir.dt.float32
F16 = mybir.dt.float16
I16 = mybir.dt.int16

USE_TRICK_EXP = True

# exp(x) ~= bitcast_fp16(int16(round(A16*x + B16)))
_SIGMA = 0.0455
A16 = 1024.0 / float(np.log(2.0))
B16 = (15.0 - _SIGMA) * 1024.0

# (col_offset, width) of each input DMA; first two small so compute starts
# after ~0.25MB instead of 1MB. All chunks stay resident in SBUF (32KB per
# partition total), so every input DMA issues with no WAR wait and the
# in-order Sync queue never head-of-line blocks an input load behind an
# output store.
X_CHUNKS = [
    (0, 1024), (1024, 1024), (2048, 2048),
    (4096, 4096), (8192, 4096), (12288, 2048), (14336, 2048),
]
# compute tiles (col, width): small first tiles shorten the lead-in, small
# last tiles shorten the serial DVE->MM->Ln->DMA drain tail.
C_TILES = [(0, 1024), (1024, 1024)] + [
    (c, 2048) for c in range(2048, 14336, 2048)
] + [(14336, 1024), (15360, 1024)]


def build_kernel(nc: bass.Bass, repeat: int = 1):
    x_d = nc.dram_tensor("x", [128, TOTAL_FREE], F16, kind="ExternalInput").ap()
    wblk_d = nc.dram_tensor("w_blk", [128, 128], F16, kind="ExternalInput").ap()
    out_d = nc.dram_tensor("out", [128, TOTAL_FREE], F16, kind="ExternalOutput").ap()

    with tile.TileContext(nc) as tc, ExitStack() as ctx:
        const_pool = ctx.enter_context(tc.tile_pool(name="const", bufs=1))
        # SBUF is ~64KB/partition usable; pool slots are sized to the largest
        # tile, so segregate x chunks by width to stay within budget.
        x_pools = {}
        for width in sorted({w for _, w in X_CHUNKS}):
            n = sum(1 for _, w in X_CHUNKS if w == width)
            x_pools[width] = ctx.enter_context(
                tc.tile_pool(name=f"x{width}", bufs=n)
            )
        p_pool = ctx.enter_context(tc.tile_pool(name="p", bufs=2))
        o_pool = ctx.enter_context(tc.tile_pool(name="o", bufs=4))
        # psO [128,2048] fp32 = 4 banks; 2 bufs = all 8 banks
        psO_pool = ctx.enter_context(tc.tile_pool(name="psO", bufs=2, space="PSUM"))

        def load_chunk(c):
            off, width = X_CHUNKS[c]
            xt = x_pools[width].tile([128, width], F16, tag=f"x{c}")
            nc.sync.dma_start(xt[:], x_d[:, off : off + width])
            return xt

        # x chunk 0 first (it gates the first DVE), then wblk, then the rest.
        xc0 = load_chunk(0)
        wblk = const_pool.tile([128, 128], F16, tag="wblk")
        nc.sync.dma_start(wblk[:], wblk_d)

        dummy_w = const_pool.tile([128, 8], mybir.dt.bfloat16, tag="dummyw")
        nc.gpsimd.memset(dummy_w[:], 1.0)

        # tiny dummy activation up front: forces the ACT table load to
        # overlap the first x DMA instead of sitting on the critical path
        warm_pool = ctx.enter_context(tc.tile_pool(name="warm", bufs=1))
        warm = warm_pool.tile([128, 1], F32, tag="warm")
        nc.scalar.activation(
            warm[:], dummy_w[:, 0:1], mybir.ActivationFunctionType.Ln
        )
        if not USE_TRICK_EXP:
            nc.scalar.activation(
                warm[:], dummy_w[:, 0:1], mybir.ActivationFunctionType.Exp
            )

        def chunk_of(col):
            for ci, (off, width) in enumerate(X_CHUNKS):
                if off <= col < off + width:
                    return ci, col - off
            raise AssertionError(col)

        for _rep in range(repeat):
            chunk_bufs = {0: xc0 if _rep == 0 else load_chunk(0)}
            for ci in range(1, len(X_CHUNKS)):
                chunk_bufs[ci] = load_chunk(ci)

            for col, width in C_TILES:
                ci, coff = chunk_of(col)
                xt = chunk_bufs[ci]
                if USE_TRICK_EXP:
                    pt = p_pool.tile([128, width], I16)
                    nc.vector.tensor_scalar(
                        pt[:],
                        xt[:, coff : coff + width],
                        A16,
                        B16,
                        op0=mybir.AluOpType.mult,
                        op1=mybir.AluOpType.add,
                    )
                    ptv = pt[:].bitcast(F16)
                else:
                    pt = p_pool.tile([128, width], F16)
                    nc.scalar.activation(
                        pt[:],
                        xt[:, coff : coff + width],
                        mybir.ActivationFunctionType.Exp,
                    )
                    ptv = pt[:]
                psO = psO_pool.tile([128, width], F32)
                for j in range(width // 128):
                    nc.tensor.matmul(
                        psO[:, bass.ts(j, 128)],
                        ptv[:, bass.ts(j, 128)],
                        wblk[:],
                        start=(j % 4 == 0),
                        stop=(j % 4 == 3),
                    )
                ot = o_pool.tile([128, width], F16)
                nc.scalar.activation(
                    ot[:], psO[:], mybir.ActivationFunctionType.Ln
                )
                # Early out stores go on the second HWDGE queue (ACT engine):
                # on a single queue the first store's packets sit behind the
                # whole input backlog (FIFO), which stalled ln_t3+ on ot
                # reuse. Late stores (once the input loads have drained) go
                # back on the Sync ring so their ~0.6us issue cost doesn't
                # serialize with Ln on the ACT queue.
                if col < 8192:
                    nc.scalar.dma_start(out_d[:, col : col + width], ot[:])
                else:
                    nc.sync.dma_start(out_d[:, col : col + width], ot[:])
    return nc


# walrus rejects >1 embedded sync-wait on engine-instruction structs
# (Matmult/Activation/DMA...). The NX sequencer executes embedded waits in
# stream order anyway, so spilling all-but-one wait onto dedicated nops
# immediately before the instruction is semantically identical.
_SPLIT_TYPES = (
    "InstMatmult",
    "InstLdweights",
    "InstActivation",
    "InstDMACopy",
    "InstMemset",
    "InstTensorTensor",
    "InstTensorScalarPtr",
    "InstCopy",
    "InstTensorReduce",
    "InstDrain",
    "InstNoOp",
)


def _split_embedded_waits(nc: bass.Bass):
    for fn in nc.m.functions:
        for blk in fn.blocks:
            insts = blk.instructions
            out = []
            for inst in insts:
                si = inst.sync_info
                if (
                    si is not None
                    and si.on_wait
                    and len(si.on_wait) > 1
                    and type(inst).__name__ in _SPLIT_TYPES
                ):
                    waits = list(si.on_wait)
                    for i, w in enumerate(waits[:-1]):
                        nop = mybir.InstNoOp(
                            name=f"{inst.name}-sw{i}",
                            engine=inst.engine,
                            sync_info=mybir.SyncInfo(on_wait=[w], on_update=[]),
                            bass_nofuse=True,
                        )
                        out.append(nop)
                    inst.sync_info = mybir.SyncInfo(
                        on_wait=[waits[-1]], on_update=list(si.on_update)
                    )
                out.append(inst)
            if len(out) != len(insts):
                blk.instructions[:] = out


def _host_weights(accumulators: np.ndarray) -> np.ndarray:
    """log_softmax over c of [1,1,Cin,S] accumulators -> exp -> block-diag."""
    acc = np.asarray(accumulators, dtype=np.float64)[0, 0]      # [Cin, S]
    m = acc.max(axis=0, keepdims=True)
    e = np.exp(acc - m)
    w = (e / e.sum(axis=0, keepdims=True)).astype(np.float16)   # [Cin, S]
    w_blk = np.zeros((128, 128), dtype=np.float16)
    for g in range(4):
        w_blk[32 * g : 32 * g + 32, 32 * g : 32 * g + 32] = w
    return w_blk


def make_in_maps(x: np.ndarray, acc: np.ndarray) -> list[dict]:
    x16 = np.asarray(x).astype(np.float16)
    w_blk = _host_weights(np.asarray(acc, dtype=np.float32))
    in_maps = []
    for c in range(N_CORES):
        xc = x16[c * B_PER_CORE : (c + 1) * B_PER_CORE].reshape(-1, C_IN)
        # quad-row transpose: xT[(row%4)*32 + ch, row//4]
        xT = np.ascontiguousarray(
            xc.reshape(TOTAL_FREE, 4, C_IN).transpose(1, 2, 0).reshape(
                128, TOTAL_FREE
            )
        )
        in_maps.append({"x": xT, "w_blk": w_blk})
    return in_maps


def unpack_out(o: np.ndarray) -> np.ndarray:
    """[128, 16384] quad-row-major fp16 -> [B_PER_CORE, H, W, S] fp32."""
    # out[p, t*2048 + j*128 + g*32 + s] holds row 4*(t*2048 + j*128 + p)+g
    o = o.reshape(128, TOTAL_FREE // 128, 4, 32)
    # axes: [p, qblk, g, s] where q = qblk*128 + p, row = 4q + g
    o = o.transpose(1, 0, 2, 3)  # [qblk, p, g, s] -> rows ascending
    return o.reshape(B_PER_CORE, H, W, N_SUMS).astype(np.float32)


_CACHE: dict = {}


def make_bass():
    return bass.Bass("TRN2", debug=False, num_swdge_queues=4)


def get_nc():
    if "nc" not in _CACHE:
        nc = build_kernel(make_bass())
        # HW path only: CoreSim can't digest post-hoc inserted nops
        _split_embedded_waits(nc)
        _CACHE["nc"] = nc
    return _CACHE["nc"]


def kernel(**inputs: np.ndarray) -> np.ndarray:
    from concourse.bass_utils import run_bass_kernel_spmd

    in_maps = make_in_maps(inputs["x"], inputs["accumulators"])
    nc = get_nc()
    res = run_bass_kernel_spmd(nc, in_maps, core_ids=list(range(N_CORES)))
    outs = [unpack_out(np.asarray(res.results[c]["out"])) for c in range(N_CORES)]
    return np.concatenate(outs, axis=0)
